# revision 1
# baseline (speedup 1.0000x reference)
"""Routed-LoRA linear layer (moe_routing) on 8 trn2 NeuronCores.

Math (per token t):
  out[t, :] = W @ x[t] + b + 2.0 * sum_n mask[n, t] * (B_n @ (A_n @ x[t]))

Strategy:
  - Data-parallel over B*T = 65536 tokens: 8192 tokens per core.
  - All operand transposes are done host-side (numpy marshaling) so the
    device only ever streams contiguous, partition-friendly layouts:
      xt  [D_IN, TOK]  = x-shard transposed      (contraction dim major)
      wt  [D_IN, D_OUT] = W.T
      at  [D_IN, NR]    = fused-A.T
      btr [NR, D_OUT]   = fused-B.T
      msk [NR, TOK]     = routing mask expanded to rank dim, pre-scaled
  - fp32r matmuls (full PE rate at N=512), LoRA delta accumulated into the
    same PSUM bank as the base matmul, bias added during PSUM->SBUF copy.
"""

import numpy as np

import concourse.bass as bass
from concourse import bacc
import concourse.mybir as mybir
import concourse.tile as tile
from concourse.bass_utils import run_bass_kernel_spmd

N_CORES = 8
B, T = 8, 8192
D_IN = 1024
D_OUT = 1024
N_ADAPT, R = 4, 16
NR = N_ADAPT * R  # 64
SCALING = 32.0 / 16.0

TOK = B * T // N_CORES  # 8192 tokens per core
SUP = 512               # tokens per supertile
N_SUP = TOK // SUP      # 16
SUB = 128               # tokens per matmul M-tile
N_SUB = SUP // SUB      # 4
P = 128
KC = D_IN // P          # 8 contraction chunks
NB = D_OUT // 512       # 2 PSUM-bank column halves

F32 = mybir.dt.float32
F32R = mybir.dt.float32r


def build_bass(nrep=1, xp_bufs=3, pso_bufs=2, n_inner=False, split_bias=False):
    nc = bacc.Bacc(
        "TRN2", target_bir_lowering=False, debug=False, num_devices=N_CORES
    )

    xt_d = nc.dram_tensor("xt", [D_IN, TOK], F32R, kind="ExternalInput")
    wt_d = nc.dram_tensor("wt", [D_IN, D_OUT], F32R, kind="ExternalInput")
    at_d = nc.dram_tensor("at", [D_IN, NR], F32R, kind="ExternalInput")
    bt_d = nc.dram_tensor("btr", [NR, D_OUT], F32R, kind="ExternalInput")
    bias_d = nc.dram_tensor("bias", [D_OUT], F32, kind="ExternalInput")
    msk_d = nc.dram_tensor("msk", [NR, TOK], F32, kind="ExternalInput")
    out_d = nc.dram_tensor("out", [TOK, D_OUT], F32, kind="ExternalOutput")

    xt_r = xt_d.ap().rearrange("(kc p) t -> p kc t", p=P)
    wt_r = wt_d.ap().rearrange("(kc p) n -> p kc n", p=P)
    at_r = at_d.ap().rearrange("(kc p) j -> p kc j", p=P)
    out_r = out_d.ap().rearrange("(s q p) n -> s p q n", q=N_SUB, p=P)
    bias_bcast = bass.AP(
        tensor=bias_d, offset=0, ap=[[0, P], [1, D_OUT]]
    )

    with tile.TileContext(nc) as tc:
        with (
            tc.tile_pool(name="const", bufs=1) as const,
            tc.tile_pool(name="xp", bufs=xp_bufs) as xp,
            tc.tile_pool(name="sp", bufs=2) as sp,
            tc.tile_pool(name="op", bufs=2) as op,
            tc.tile_pool(name="pss", bufs=2, space="PSUM") as pss,
            tc.tile_pool(name="pso", bufs=pso_bufs, space="PSUM") as pso,
        ):
            w_sb = const.tile([P, KC, D_OUT], F32R)
            a_sb = const.tile([P, KC, NR], F32R)
            bt_sb = const.tile([NR, D_OUT], F32R)
            b_sb = const.tile([P, D_OUT], F32)
            m_sb = const.tile([NR, TOK], F32)
            # Preload order matters for startup latency: the first s-pass
            # matmuls need a_sb + x0 (sync queue), the first main matmuls
            # need w chunk 0 (scalar queue, split per-k so MMs start after
            # ~1.4us instead of waiting for the whole 4MB W load).
            nc.sync.dma_start(out=a_sb[:], in_=at_r)
            nc.scalar.dma_start(out=bt_sb[:], in_=bt_d.ap())
            for k in range(KC):
                nc.scalar.dma_start(out=w_sb[:, k, :], in_=wt_r[:, k, :])
            nc.gpsimd.dma_start(out=b_sb[:], in_=bias_bcast)

            for _rep in range(nrep):
                for s in range(N_SUP):
                    t0 = s * SUP
                    x_sb = xp.tile([P, KC, SUP], F32R, tag="x")
                    nc.sync.dma_start(
                        out=x_sb[:], in_=xt_r[:, :, t0 : t0 + SUP]
                    )
                    if _rep == 0:
                        # stream the mask in per-supertile so the first
                        # mask-multiply isn't gated on a monolithic 2MB load
                        nc.sync.dma_start(
                            out=m_sb[:, t0 : t0 + SUP],
                            in_=msk_d.ap()[:, t0 : t0 + SUP],
                        )

                    # s.T = fused_A @ x.T for this supertile: [NR, SUP]
                    s_ps = pss.tile([NR, SUP], F32, tag="sps")
                    for k in range(KC):
                        nc.tensor.matmul(
                            s_ps[:],
                            a_sb[:, k, :],
                            x_sb[:, k, :],
                            start=(k == 0),
                            stop=(k == KC - 1),
                        )
                    sm_sb = sp.tile([NR, SUP], F32R, tag="sm")
                    nc.vector.tensor_mul(
                        sm_sb[:], s_ps[:], m_sb[:, t0 : t0 + SUP]
                    )

                    o_sb = op.tile([P, N_SUB, D_OUT], F32, tag="o")
                    for q in range(N_SUB):
                        ts = q * SUB
                        o_ps = pso.tile([P, D_OUT], F32, tag="ops")
                        if n_inner:
                            for k in range(KC):
                                for n in range(NB):
                                    nsl = slice(n * 512, (n + 1) * 512)
                                    nc.tensor.matmul(
                                        o_ps[:, nsl],
                                        x_sb[:, k, ts : ts + SUB],
                                        w_sb[:, k, nsl],
                                        start=(k == 0),
                                        stop=False,
                                        skip_group_check=True,
                                    )
                            for n in range(NB):
                                nsl = slice(n * 512, (n + 1) * 512)
                                nc.tensor.matmul(
                                    o_ps[:, nsl],
                                    sm_sb[:, ts : ts + SUB],
                                    bt_sb[:, nsl],
                                    start=False,
                                    stop=True,
                                    skip_group_check=True,
                                )
                        else:
                            for n in range(NB):
                                nsl = slice(n * 512, (n + 1) * 512)
                                for k in range(KC):
                                    nc.tensor.matmul(
                                        o_ps[:, nsl],
                                        x_sb[:, k, ts : ts + SUB],
                                        w_sb[:, k, nsl],
                                        start=(k == 0),
                                        stop=False,
                                    )
                                nc.tensor.matmul(
                                    o_ps[:, nsl],
                                    sm_sb[:, ts : ts + SUB],
                                    bt_sb[:, nsl],
                                    start=False,
                                    stop=True,
                                )
                        if split_bias:
                            for n in range(NB):
                                nsl = slice(n * 512, (n + 1) * 512)
                                nc.vector.tensor_add(
                                    o_sb[:, q, nsl], o_ps[:, nsl], b_sb[:, nsl]
                                )
                        else:
                            nc.vector.tensor_add(o_sb[:, q, :], o_ps[:], b_sb[:])
                    nc.scalar.dma_start(out=out_r[s], in_=o_sb[:])

    nc.compile()
    return nc


_NC_CACHE = None


def _get_nc():
    global _NC_CACHE
    if _NC_CACHE is None:
        _NC_CACHE = build_bass()
    return _NC_CACHE


def make_in_maps(x, W, b, lora_A, lora_B, masks):
    x = np.ascontiguousarray(x, dtype=np.float32)
    W = np.ascontiguousarray(W, dtype=np.float32)
    b = np.ascontiguousarray(b, dtype=np.float32)
    lora_A = np.ascontiguousarray(lora_A, dtype=np.float32)
    lora_B = np.ascontiguousarray(lora_B, dtype=np.float32)
    masks = np.ascontiguousarray(masks, dtype=np.float32)

    x_flat = x.reshape(B * T, D_IN)
    A_flat = lora_A.reshape(NR, D_IN)
    B_flat = lora_B.transpose(1, 0, 2).reshape(D_OUT, NR)

    wt = np.ascontiguousarray(W.T)            # [D_IN, D_OUT]
    at = np.ascontiguousarray(A_flat.T)       # [D_IN, NR]
    btr = np.ascontiguousarray(B_flat.T)      # [NR, D_OUT]

    m_full = masks[..., 0].reshape(N_ADAPT, B * T) * np.float32(SCALING)
    m_exp = np.repeat(m_full, R, axis=0)      # [NR, B*T]

    in_maps = []
    for c in range(N_CORES):
        sl = slice(c * TOK, (c + 1) * TOK)
        in_maps.append(
            {
                "xt": np.ascontiguousarray(x_flat[sl].T),
                "wt": wt,
                "at": at,
                "btr": btr,
                "bias": b,
                "msk": np.ascontiguousarray(m_exp[:, sl]),
            }
        )
    return in_maps


def kernel(x, W, b, lora_A, lora_B, masks):
    nc = _get_nc()
    in_maps = make_in_maps(x, W, b, lora_A, lora_B, masks)
    res = run_bass_kernel_spmd(nc, in_maps, core_ids=list(range(N_CORES)))
    out = np.concatenate([r["out"] for r in res.results], axis=0)
    out = out.reshape(B, T, D_OUT)
    return out



# revision 6
# speedup vs baseline: 1.4552x; 1.4552x over previous
"""Routed-LoRA linear layer (moe_routing) on 8 trn2 NeuronCores.

Math (per token t):
  out[t, :] = W @ x[t] + b + 2.0 * sum_n mask[n, t] * (B_n @ (A_n @ x[t]))

Strategy (v2, split-fp8):
  - Data-parallel over B*T = 65536 tokens: 8192 tokens per core.
  - Output computed TRANSPOSED on-device: out.T [D_OUT, TOK], so the bias is
    per-partition and the Activation engine can do the fused
    `out = psum*scale + bias` eviction (DVE stays nearly idle).
  - Base matmul in split fp8 (e4m3) with DoubleRow perf mode:
      x/sx   = x1 + x2   (x2 quantizes the residual at the same scale)
      W.T/sw = W1 + W2
      x@W.T/(sx*sw) ~= W1.T'x1 + W2.T'x1 + W1.T'x2   (x2@W2 dropped)
    DoubleRow contracts 2 k-subtiles (K=256) per matmul at 0.5 cycles/row.
  - LoRA s-pass s = A@x.T from (A1, x1) fp8 DoubleRow; mask multiply on DVE
    with all scale constants folded into the mask host-side; delta matmul in
    fp32r accumulated into the same PSUM bank as the base terms.
  - Output in bf16 (host casts back to fp32): halves output DMA traffic.
    Measured end-to-end max rel err ~7.8e-3 (budget 2e-2).
"""

import numpy as np
import ml_dtypes

import concourse.bass as bass
from concourse import bacc
import concourse.mybir as mybir
import concourse.tile as tile
from concourse.bass_utils import run_bass_kernel_spmd

N_CORES = 8
B, T = 8, 8192
D_IN = 1024
D_OUT = 1024
N_ADAPT, R = 4, 16
NR = N_ADAPT * R  # 64
SCALING = 32.0 / 16.0

TOK = B * T // N_CORES  # 8192 tokens per core
SUP = 512               # tokens per supertile
N_SUP = TOK // SUP      # 16
P = 128
KC = D_IN // P          # 8 contraction chunks of 128
KD = KC // 2            # 4 DoubleRow chunks of 256
NCC = D_OUT // P        # 8 output-row chunks of 128

SX = 2.0 ** -5          # x scale
SW = 2.0 ** -10         # W scale
SA = 2.0 ** -10         # lora_A scale
OSCALE = SX * SW        # psum -> out scale
MSCALE = SCALING * SA / SW  # folded into the routing mask (= 2.0, fp8-exact)

F32 = mybir.dt.float32
F32R = mybir.dt.float32r
BF16 = mybir.dt.bfloat16
FP8 = mybir.dt.float8e4
DR = mybir.MatmulPerfMode.DoubleRow
E4 = ml_dtypes.float8_e4m3


def build_bass(n_warm=10):
    nc = bacc.Bacc(
        "TRN2", target_bir_lowering=False, debug=False, num_devices=N_CORES
    )

    x1_d = nc.dram_tensor("x1", [D_IN, TOK], FP8, kind="ExternalInput")
    x2_d = nc.dram_tensor("x2", [D_IN, TOK], FP8, kind="ExternalInput")
    w1_d = nc.dram_tensor("w1", [D_IN, D_OUT], FP8, kind="ExternalInput")
    w2_d = nc.dram_tensor("w2", [D_IN, D_OUT], FP8, kind="ExternalInput")
    a1_d = nc.dram_tensor("a1", [D_IN, NR], FP8, kind="ExternalInput")
    btr_d = nc.dram_tensor("btr", [NR, D_OUT], F32R, kind="ExternalInput")
    bias_d = nc.dram_tensor("bias", [P, NCC], F32, kind="ExternalInput")
    msk_d = nc.dram_tensor("msk", [NR, TOK], FP8, kind="ExternalInput")
    out_d = nc.dram_tensor("out", [D_OUT, TOK], BF16, kind="ExternalOutput")

    x1_r = x1_d.ap().rearrange("(kc p) t -> p kc t", p=P)
    x2_r = x2_d.ap().rearrange("(kc p) t -> p kc t", p=P)
    w1_r = w1_d.ap().rearrange("(kc p) n -> p kc n", p=P)
    w2_r = w2_d.ap().rearrange("(kc p) n -> p kc n", p=P)
    a1_r = a1_d.ap().rearrange("(kc p) j -> p kc j", p=P)
    out_r = out_d.ap().rearrange("(ncc p) t -> p ncc t", p=P)

    with tile.TileContext(nc) as tc:
        with (
            tc.tile_pool(name="const", bufs=1) as const,
            tc.tile_pool(name="xp", bufs=3) as xp,
            tc.tile_pool(name="smp", bufs=2) as smp,
            tc.tile_pool(name="op", bufs=2) as op,
            tc.tile_pool(name="pss", bufs=2, space="PSUM") as pss,
            tc.tile_pool(name="pso", bufs=3, space="PSUM") as pso,
            tc.tile_pool(name="pwu", bufs=1, space="PSUM") as pwu,
        ):
            w1_sb = const.tile([P, KC, D_OUT], FP8)
            w2_sb = const.tile([P, KC, D_OUT], FP8)
            a1_sb = const.tile([P, KC, NR], FP8)
            btr_sb = const.tile([NR, D_OUT], F32R)
            b_sb = const.tile([P, NCC], F32)
            m_sb = const.tile([NR, TOK], FP8)
            wu_sb = const.tile([P, SUP], BF16)

            # Critical-path preloads, ordered so the first supertile's
            # matmuls can start ASAP: a1 + x1[0] feed the s-pass, w1 k-chunks
            # feed the base term-1 matmuls.  Everything later (w2, btr, ...)
            # arrives while term-1 runs.
            nc.sync.dma_start(out=a1_sb[:], in_=a1_r)
            for k in range(KC):
                nc.scalar.dma_start(out=w1_sb[:, k, :], in_=w1_r[:, k, :])
            nc.scalar.dma_start(out=w2_sb[:], in_=w2_r)
            nc.gpsimd.dma_start(out=m_sb[:], in_=msk_d.ap())
            nc.gpsimd.dma_start(out=btr_sb[:], in_=btr_d.ap())
            nc.gpsimd.dma_start(out=b_sb[:], in_=bias_d.ap())

            # PE warmup: ramp the p-state model during the preload DMAs so
            # the first real matmuls run at full clock.  Reads a tile we
            # memset first (never consumed downstream).
            if n_warm:
                nc.vector.memset(wu_sb[:], 0.0)
                wu_ps = pwu.tile([P, SUP], F32)
                for i in range(n_warm):
                    nc.tensor.matmul(
                        wu_ps[:],
                        wu_sb[:, 0:P],
                        wu_sb[:],
                        start=(i == 0),
                        stop=(i == n_warm - 1),
                    )

            for s in range(N_SUP):
                t0 = s * SUP
                x1_sb = xp.tile([P, KC, SUP], FP8, tag="x1")
                nc.sync.dma_start(out=x1_sb[:], in_=x1_r[:, :, t0 : t0 + SUP])
                x2_sb = xp.tile([P, KC, SUP], FP8, tag="x2")
                nc.sync.dma_start(out=x2_sb[:], in_=x2_r[:, :, t0 : t0 + SUP])

                # s.T = fused_A @ x.T for this supertile: [NR, SUP]
                s_ps = pss.tile([NR, SUP], F32, tag="sps")
                for kd in range(KD):
                    nc.tensor.matmul(
                        s_ps[:],
                        a1_sb[:, 2 * kd : 2 * kd + 2, :],
                        x1_sb[:, 2 * kd : 2 * kd + 2, :],
                        start=(kd == 0),
                        stop=(kd == KD - 1),
                        perf_mode=DR,
                    )
                sm_sb = smp.tile([NR, SUP], F32R, tag="sm")
                nc.vector.tensor_mul(
                    sm_sb[:], s_ps[:], m_sb[:, t0 : t0 + SUP]
                )

                o_sb = op.tile([P, NCC, SUP], BF16, tag="o")
                for j in range(NCC):
                    nsl = slice(j * P, (j + 1) * P)
                    o_ps = pso.tile([P, SUP], F32, tag="ops")
                    for w_sb, x_sb, first in (
                        (w1_sb, x1_sb, True),
                        (w2_sb, x1_sb, False),
                        (w1_sb, x2_sb, False),
                    ):
                        for kd in range(KD):
                            nc.tensor.matmul(
                                o_ps[:],
                                w_sb[:, 2 * kd : 2 * kd + 2, nsl],
                                x_sb[:, 2 * kd : 2 * kd + 2, :],
                                start=(first and kd == 0),
                                stop=False,
                                perf_mode=DR,
                            )
                    nc.tensor.matmul(
                        o_ps[:],
                        btr_sb[:, nsl],
                        sm_sb[:],
                        start=False,
                        stop=True,
                    )
                    nc.scalar.activation(
                        o_sb[:, j, :],
                        o_ps[:],
                        mybir.ActivationFunctionType.Identity,
                        bias=b_sb[:, j : j + 1],
                        scale=float(OSCALE),
                    )
                if s < N_SUP - 1:
                    nc.scalar.dma_start(
                        out=out_r[:, :, t0 : t0 + SUP], in_=o_sb[:]
                    )
                else:
                    # last supertile: evict per-chunk so the tail is one
                    # chunk long, not the whole supertile
                    for j in range(NCC):
                        nc.sync.dma_start(
                            out=out_r[:, j, t0 : t0 + SUP], in_=o_sb[:, j, :]
                        )

    nc.compile()
    return nc


_NC_CACHE = None


def _get_nc():
    global _NC_CACHE
    if _NC_CACHE is None:
        _NC_CACHE = build_bass()
    return _NC_CACHE


def _q8(v):
    return np.asarray(v, dtype=E4)


def make_in_maps(x, W, b, lora_A, lora_B, masks):
    x = np.asarray(x, dtype=np.float32)
    W = np.asarray(W, dtype=np.float32)
    b = np.ascontiguousarray(np.asarray(b, dtype=np.float32))
    lora_A = np.asarray(lora_A, dtype=np.float32)
    lora_B = np.asarray(lora_B, dtype=np.float32)
    masks = np.asarray(masks, dtype=np.float32)

    xs = np.ascontiguousarray(x.reshape(B * T, D_IN).T) * np.float32(1.0 / SX)
    x1 = _q8(xs)
    x2 = _q8(xs - x1.astype(np.float32))

    ws = np.ascontiguousarray(W.T) * np.float32(1.0 / SW)  # [D_IN, D_OUT]
    w1 = _q8(ws)
    w2 = _q8(ws - w1.astype(np.float32))

    A_flat = lora_A.reshape(NR, D_IN)
    a1 = _q8(np.ascontiguousarray(A_flat.T) * np.float32(1.0 / SA))

    B_flat = lora_B.transpose(1, 0, 2).reshape(D_OUT, NR)
    btr = np.ascontiguousarray(B_flat.T)      # [NR, D_OUT]

    bias = np.ascontiguousarray(b.reshape(NCC, P).T)  # [P, NCC]

    m_full = masks[..., 0].reshape(N_ADAPT, B * T) * np.float32(MSCALE)
    m_exp = _q8(np.repeat(m_full, R, axis=0))  # [NR, B*T], values {0, 2.0}

    in_maps = []
    for c in range(N_CORES):
        sl = slice(c * TOK, (c + 1) * TOK)
        in_maps.append(
            {
                "x1": np.ascontiguousarray(x1[:, sl]),
                "x2": np.ascontiguousarray(x2[:, sl]),
                "w1": w1,
                "w2": w2,
                "a1": a1,
                "btr": btr,
                "bias": bias,
                "msk": np.ascontiguousarray(m_exp[:, sl]),
            }
        )
    return in_maps


def kernel(x, W, b, lora_A, lora_B, masks):
    nc = _get_nc()
    in_maps = make_in_maps(x, W, b, lora_A, lora_B, masks)
    res = run_bass_kernel_spmd(nc, in_maps, core_ids=list(range(N_CORES)))
    out = np.empty((B * T, D_OUT), dtype=np.float32)
    for c, r in enumerate(res.results):
        out[c * TOK : (c + 1) * TOK, :] = r["out"].astype(np.float32).T
    return out.reshape(B, T, D_OUT)


# revision 22
# speedup vs baseline: 1.5107x; 1.0381x over previous
"""Routed-LoRA linear layer (moe_routing) on 8 trn2 NeuronCores.

Math (per token t):
  out[t, :] = W @ x[t] + b + 2.0 * sum_n mask[n, t] * (B_n @ (A_n @ x[t]))

Strategy (v2, split-fp8):
  - Data-parallel over B*T = 65536 tokens: 8192 tokens per core.
  - Output computed TRANSPOSED on-device: out.T [D_OUT, TOK], so the bias is
    per-partition and the Activation engine can do the fused
    `out = psum*scale + bias` eviction (DVE stays nearly idle).
  - Base matmul in split fp8 (e4m3) with DoubleRow perf mode:
      x/sx   = x1 + x2   (x2 quantizes the residual at the same scale)
      W.T/sw = W1 + W2
      x@W.T/(sx*sw) ~= W1.T'x1 + W2.T'x1 + W1.T'x2   (x2@W2 dropped)
    DoubleRow contracts 2 k-subtiles (K=256) per matmul at 0.5 cycles/row.
  - LoRA s-pass s = A@x.T from (A1, x1) fp8 DoubleRow; mask multiply on DVE
    with all scale constants folded into the mask host-side; delta matmul in
    fp32r accumulated into the same PSUM bank as the base terms.
  - Output in bf16 (host casts back to fp32): halves output DMA traffic.
    Measured end-to-end max rel err ~7.8e-3 (budget 2e-2).
"""

import numpy as np
import ml_dtypes

import concourse.bass as bass
from concourse import bacc
import concourse.mybir as mybir
import concourse.tile as tile
from concourse.bass_utils import run_bass_kernel_spmd

N_CORES = 8
B, T = 8, 8192
D_IN = 1024
D_OUT = 1024
N_ADAPT, R = 4, 16
NR = N_ADAPT * R  # 64
SCALING = 32.0 / 16.0

TOK = B * T // N_CORES  # 8192 tokens per core
SUP = 512               # tokens per supertile
N_SUP = TOK // SUP      # 16
P = 128
KC = D_IN // P          # 8 contraction chunks of 128
KD = KC // 2            # 4 DoubleRow chunks of 256
NCC = D_OUT // P        # 8 output-row chunks of 128

SX = 2.0 ** -5          # x scale
SW = 2.0 ** -10         # W scale
SA = 2.0 ** -10         # lora_A scale
SB = 2.0 ** -11         # lora_B scale
OSCALE = SX * SW        # psum -> out scale
MSCALE = SCALING * SA * SB / SW  # folded into the routing mask (2^-10, bf16-exact)

F32 = mybir.dt.float32
F32R = mybir.dt.float32r
BF16 = mybir.dt.bfloat16
FP8 = mybir.dt.float8e4
DR = mybir.MatmulPerfMode.DoubleRow
E4 = ml_dtypes.float8_e4m3


def build_bass(n_warm=4, pre=1, pso_bufs=4, w1_split=2, w2_split=2):
    nc = bacc.Bacc(
        "TRN2", target_bir_lowering=False, debug=False, num_devices=N_CORES
    )

    x1_d = nc.dram_tensor("x1", [D_IN, TOK], FP8, kind="ExternalInput")
    x2_d = nc.dram_tensor("x2", [D_IN, TOK], FP8, kind="ExternalInput")
    w1_d = nc.dram_tensor("w1", [D_IN, D_OUT], FP8, kind="ExternalInput")
    w2_d = nc.dram_tensor("w2", [D_IN, D_OUT], FP8, kind="ExternalInput")
    a1_d = nc.dram_tensor("a1", [D_IN, NR], FP8, kind="ExternalInput")
    btr_d = nc.dram_tensor("btr", [NR // 2, 2 * D_OUT], FP8, kind="ExternalInput")
    bias_d = nc.dram_tensor("bias", [P, NCC], F32, kind="ExternalInput")
    msk_d = nc.dram_tensor("msk", [NR, TOK], BF16, kind="ExternalInput")
    out_d = nc.dram_tensor("out", [D_OUT, TOK], BF16, kind="ExternalOutput")

    x1_r = x1_d.ap().rearrange("(kc p) t -> p kc t", p=P)
    x2_r = x2_d.ap().rearrange("(kc p) t -> p kc t", p=P)
    w1_r = w1_d.ap().rearrange("(kc p) n -> p kc n", p=P)
    w2_r = w2_d.ap().rearrange("(kc p) n -> p kc n", p=P)
    a1_r = a1_d.ap().rearrange("(kc p) j -> p kc j", p=P)
    btr_r = btr_d.ap().rearrange("r (i n) -> r i n", n=D_OUT)
    msk_r = msk_d.ap().rearrange("r (i t) -> r i t", t=TOK)
    out_r = out_d.ap().rearrange("(ncc p) t -> p ncc t", p=P)

    with tile.TileContext(nc) as tc:
        with (
            tc.tile_pool(name="const", bufs=1) as const,
            tc.tile_pool(name="xp", bufs=3) as xp,
            tc.tile_pool(name="smp", bufs=2) as smp,
            tc.tile_pool(name="op", bufs=2) as op,
            tc.tile_pool(name="pss", bufs=2, space="PSUM") as pss,
            tc.tile_pool(name="pso", bufs=pso_bufs, space="PSUM") as pso,
        ):
            w1_sb = const.tile([P, KC, D_OUT], FP8)
            w2_sb = const.tile([P, KC, D_OUT], FP8)
            a1_sb = const.tile([P, KC, NR], FP8)
            btr_sb = const.tile([NR // 2, 2, D_OUT], FP8)
            b_sb = const.tile([P, NCC], F32)
            m_sb = const.tile([NR // 2, 2, TOK], BF16)
            wu_sb = const.tile([P, SUP], BF16)

            # Critical-path preloads, ordered so the first supertile's
            # matmuls can start ASAP: a1 + x1[0] feed the s-pass, w1 k-chunks
            # feed the base term-1 matmuls.  Everything later (w2, btr, ...)
            # arrives while term-1 runs.
            nc.sync.dma_start(out=a1_sb[:], in_=a1_r)
            kq1 = KC // w1_split
            for k0 in range(0, KC, kq1):
                nc.scalar.dma_start(
                    out=w1_sb[:, k0 : k0 + kq1, :], in_=w1_r[:, k0 : k0 + kq1, :]
                )
            kq2 = KC // w2_split
            for k0 in range(0, KC, kq2):
                nc.scalar.dma_start(
                    out=w2_sb[:, k0 : k0 + kq2, :], in_=w2_r[:, k0 : k0 + kq2, :]
                )
            nc.gpsimd.dma_start(
                out=m_sb[:, :, : TOK // 2], in_=msk_r[:, :, : TOK // 2]
            )
            nc.gpsimd.dma_start(out=btr_sb[:], in_=btr_r)
            nc.gpsimd.dma_start(out=b_sb[:], in_=bias_d.ap())
            nc.gpsimd.dma_start(
                out=m_sb[:, :, TOK // 2 :], in_=msk_r[:, :, TOK // 2 :]
            )

            # PE warmup: ramp the p-state model during the preload DMAs so
            # the first real matmuls run at full clock.  Reads a tile we
            # memset first (never consumed downstream).
            if n_warm:
                nc.vector.memset(wu_sb[:], 0.0)
                wu_ps = pso.tile([P, SUP], F32, tag="ops")
                for i in range(n_warm):
                    nc.tensor.matmul(
                        wu_ps[:],
                        wu_sb[:, 0:P],
                        wu_sb[:],
                        start=(i == 0),
                        stop=(i == n_warm - 1),
                    )

            for s in range(N_SUP):
                t0 = s * SUP
                x1_sb = xp.tile([P, KC, SUP], FP8, tag="x1")
                nc.sync.dma_start(out=x1_sb[:], in_=x1_r[:, :, t0 : t0 + SUP])
                x2_sb = xp.tile([P, KC, SUP], FP8, tag="x2")
                nc.sync.dma_start(out=x2_sb[:], in_=x2_r[:, :, t0 : t0 + SUP])

                # s.T = fused_A @ x.T for this supertile, in two 32-row
                # halves so sm can be laid out [32, 2, SUP] for a DoubleRow
                # delta matmul (rows r and r+32 pair up as the two k-subtiles)
                s_half = []
                for h in range(2):
                    s_ps = pss.tile([NR // 2, SUP], F32, tag=f"sps{h}")
                    s_half.append(s_ps)
                    for kd in range(KD):
                        nc.tensor.matmul(
                            s_ps[:],
                            a1_sb[:, 2 * kd : 2 * kd + 2, 32 * h : 32 * h + 32],
                            x1_sb[:, 2 * kd : 2 * kd + 2, :],
                            start=(kd == 0),
                            stop=(kd == KD - 1),
                            perf_mode=DR,
                        )
                sm_sb = smp.tile([NR // 2, 2, SUP], FP8, tag="sm")
                for h in range(2):
                    nc.vector.tensor_mul(
                        sm_sb[:, h, :], s_half[h][:], m_sb[:, h, t0 : t0 + SUP]
                    )

                o_sb = op.tile([P, NCC, SUP], BF16, tag="o")

                # Software-pipelined chunk groups: open chunk j's psum group
                # with its term-1 (w1*x1) matmuls up to PRE chunks ahead of
                # closing group j-PRE with its term-2/3 + delta matmuls.
                # This gives the PE w1-only work to chew on while w2/x2 (and
                # at startup btr/msk) are still in flight.
                o_pss = {}

                def open_group(j):
                    nsl = slice(j * P, (j + 1) * P)
                    o_ps = pso.tile([P, SUP], F32, tag="ops")
                    o_pss[j] = o_ps
                    for kd in range(KD):
                        nc.tensor.matmul(
                            o_ps[:],
                            w1_sb[:, 2 * kd : 2 * kd + 2, nsl],
                            x1_sb[:, 2 * kd : 2 * kd + 2, :],
                            start=(kd == 0),
                            stop=False,
                            perf_mode=DR,
                        )

                def close_group(j):
                    nsl = slice(j * P, (j + 1) * P)
                    o_ps = o_pss.pop(j)
                    for kd in range(KD):
                        nc.tensor.matmul(
                            o_ps[:],
                            w1_sb[:, 2 * kd : 2 * kd + 2, nsl],
                            x2_sb[:, 2 * kd : 2 * kd + 2, :],
                            start=False,
                            stop=False,
                            perf_mode=DR,
                        )
                    for kd in range(KD):
                        nc.tensor.matmul(
                            o_ps[:],
                            w2_sb[:, 2 * kd : 2 * kd + 2, nsl],
                            x1_sb[:, 2 * kd : 2 * kd + 2, :],
                            start=False,
                            stop=False,
                            perf_mode=DR,
                        )
                    nc.tensor.matmul(
                        o_ps[:],
                        btr_sb[:, :, nsl],
                        sm_sb[:],
                        start=False,
                        stop=True,
                        perf_mode=DR,
                    )
                    nc.scalar.activation(
                        o_sb[:, j, :],
                        o_ps[:],
                        mybir.ActivationFunctionType.Identity,
                        bias=b_sb[:, j : j + 1],
                        scale=float(OSCALE),
                    )

                PRE = pre
                for j in range(NCC):
                    open_group(j)
                    if j >= PRE:
                        close_group(j - PRE)
                for j in range(NCC - PRE, NCC):
                    close_group(j)
                if s < N_SUP - 1:
                    nc.scalar.dma_start(
                        out=out_r[:, :, t0 : t0 + SUP], in_=o_sb[:]
                    )
                else:
                    # last supertile: evict per-chunk so the tail is one
                    # chunk long, not the whole supertile
                    for j in range(NCC):
                        nc.sync.dma_start(
                            out=out_r[:, j, t0 : t0 + SUP], in_=o_sb[:, j, :]
                        )

    nc.compile()
    return nc


_NC_CACHE = None


def _get_nc():
    global _NC_CACHE
    if _NC_CACHE is None:
        _NC_CACHE = build_bass()
    return _NC_CACHE


def _q8(v):
    return np.asarray(v, dtype=E4)


def make_in_maps(x, W, b, lora_A, lora_B, masks):
    x = np.asarray(x, dtype=np.float32)
    W = np.asarray(W, dtype=np.float32)
    b = np.ascontiguousarray(np.asarray(b, dtype=np.float32))
    lora_A = np.asarray(lora_A, dtype=np.float32)
    lora_B = np.asarray(lora_B, dtype=np.float32)
    masks = np.asarray(masks, dtype=np.float32)

    xs = np.ascontiguousarray(x.reshape(B * T, D_IN).T) * np.float32(1.0 / SX)
    x1 = _q8(xs)
    x2 = _q8(xs - x1.astype(np.float32))

    ws = np.ascontiguousarray(W.T) * np.float32(1.0 / SW)  # [D_IN, D_OUT]
    w1 = _q8(ws)
    w2 = _q8(ws - w1.astype(np.float32))

    A_flat = lora_A.reshape(NR, D_IN)
    a1 = _q8(np.ascontiguousarray(A_flat.T) * np.float32(1.0 / SA))

    B_flat = lora_B.transpose(1, 0, 2).reshape(D_OUT, NR)
    btr_q = _q8(B_flat.T * np.float32(1.0 / SB))  # [NR, D_OUT] fp8
    # DoubleRow pairing: rows r and r+32 become the two k-subtiles
    btr = np.ascontiguousarray(
        btr_q.reshape(2, NR // 2, D_OUT).transpose(1, 0, 2).reshape(
            NR // 2, 2 * D_OUT
        )
    )

    bias = np.ascontiguousarray(b.reshape(NCC, P).T)  # [P, NCC]

    m_full = masks[..., 0].reshape(N_ADAPT, B * T) * np.float32(MSCALE)
    m_exp = np.repeat(m_full, R, axis=0).astype(ml_dtypes.bfloat16)  # [NR, BT]

    in_maps = []
    for c in range(N_CORES):
        sl = slice(c * TOK, (c + 1) * TOK)
        in_maps.append(
            {
                "x1": np.ascontiguousarray(x1[:, sl]),
                "x2": np.ascontiguousarray(x2[:, sl]),
                "w1": w1,
                "w2": w2,
                "a1": a1,
                "btr": btr,
                "bias": bias,
                "msk": np.ascontiguousarray(
                    m_exp[:, sl]
                    .reshape(2, NR // 2, TOK)
                    .transpose(1, 0, 2)
                    .reshape(NR // 2, 2 * TOK)
                ),
            }
        )
    return in_maps


def kernel(x, W, b, lora_A, lora_B, masks):
    nc = _get_nc()
    in_maps = make_in_maps(x, W, b, lora_A, lora_B, masks)
    res = run_bass_kernel_spmd(nc, in_maps, core_ids=list(range(N_CORES)))
    out = np.empty((B * T, D_OUT), dtype=np.float32)
    for c, r in enumerate(res.results):
        out[c * TOK : (c + 1) * TOK, :] = r["out"].astype(np.float32).T
    return out.reshape(B, T, D_OUT)


# revision 32
# speedup vs baseline: 1.6168x; 1.0702x over previous
"""Routed-LoRA linear layer (moe_routing) on 8 trn2 NeuronCores.

Math (per token t):
  out[t, :] = W @ x[t] + b + 2.0 * sum_n mask[n, t] * (B_n @ (A_n @ x[t]))

Strategy (v2, split-fp8):
  - Data-parallel over B*T = 65536 tokens: 8192 tokens per core.
  - Output computed TRANSPOSED on-device: out.T [D_OUT, TOK], so the bias is
    per-partition and the Activation engine can do the fused
    `out = psum*scale + bias` eviction (DVE stays nearly idle).
  - Base matmul in split fp8 (e4m3) with DoubleRow perf mode:
      x/sx   = x1 + x2   (x2 quantizes the residual at the same scale)
      W.T/sw = W1 + W2
      x@W.T/(sx*sw) ~= W1.T'x1 + W2.T'x1 + W1.T'x2   (x2@W2 dropped)
    DoubleRow contracts 2 k-subtiles (K=256) per matmul at 0.5 cycles/row.
  - LoRA s-pass s = A@x.T from (A1, x1) fp8 DoubleRow; mask multiply on DVE
    with all scale constants folded into the mask host-side; delta matmul in
    fp32r accumulated into the same PSUM bank as the base terms.
  - Output in bf16 (host casts back to fp32): halves output DMA traffic.
    Measured end-to-end max rel err ~7.8e-3 (budget 2e-2).
"""

import numpy as np
import ml_dtypes

import concourse.bass as bass
from concourse import bacc
import concourse.mybir as mybir
import concourse.tile as tile
from concourse.bass_utils import run_bass_kernel_spmd

N_CORES = 8
B, T = 8, 8192
D_IN = 1024
D_OUT = 1024
N_ADAPT, R = 4, 16
NR = N_ADAPT * R  # 64
SCALING = 32.0 / 16.0

TOK = B * T // N_CORES  # 8192 tokens per core
SUP = 512               # tokens per supertile
N_SUP = TOK // SUP      # 16
P = 128
KC = D_IN // P          # 8 contraction chunks of 128
KD = KC // 2            # 4 DoubleRow chunks of 256
NCC = D_OUT // P        # 8 output-row chunks of 128

SX = 2.0 ** -5          # x scale
SW = 2.0 ** -10         # W scale
SA = 2.0 ** -10         # lora_A scale
SB = 2.0 ** -11         # lora_B scale
OSCALE = SX * SW        # psum -> out scale
MSCALE = SCALING * SA * SB / SW  # folded into the routing mask (2^-10, bf16-exact)

F32 = mybir.dt.float32
F32R = mybir.dt.float32r
BF16 = mybir.dt.bfloat16
FP8 = mybir.dt.float8e4
DR = mybir.MatmulPerfMode.DoubleRow
E4 = ml_dtypes.float8_e4m3


def build_bass(n_warm=4, pre=1, pso_bufs=4, w1_split=2, w2_split=2, split_first=False, split_last=False, preload_mode=0, xp_bufs=3):
    nc = bacc.Bacc(
        "TRN2", target_bir_lowering=False, debug=False, num_devices=N_CORES
    )

    x1_d = nc.dram_tensor("x1", [D_IN, TOK], FP8, kind="ExternalInput")
    x2_d = nc.dram_tensor("x2", [D_IN, TOK], FP8, kind="ExternalInput")
    w1_d = nc.dram_tensor("w1", [D_IN, D_OUT], FP8, kind="ExternalInput")
    w2_d = nc.dram_tensor("w2", [D_IN, D_OUT], FP8, kind="ExternalInput")
    a1_d = nc.dram_tensor("a1", [D_IN, NR], FP8, kind="ExternalInput")
    btr_d = nc.dram_tensor("btr", [NR // 2, 2 * D_OUT], FP8, kind="ExternalInput")
    bias_d = nc.dram_tensor("bias", [P, NCC], F32, kind="ExternalInput")
    msk_d = nc.dram_tensor("msk", [NR, TOK], BF16, kind="ExternalInput")
    out_d = nc.dram_tensor("out", [D_OUT, TOK], BF16, kind="ExternalOutput")

    x1_r = x1_d.ap().rearrange("(kc p) t -> p kc t", p=P)
    x2_r = x2_d.ap().rearrange("(kc p) t -> p kc t", p=P)
    w1_r = w1_d.ap().rearrange("(kc p) n -> p kc n", p=P)
    w2_r = w2_d.ap().rearrange("(kc p) n -> p kc n", p=P)
    a1_r = a1_d.ap().rearrange("(kc p) j -> p kc j", p=P)
    btr_r = btr_d.ap().rearrange("r (i n) -> r i n", n=D_OUT)
    msk_r = msk_d.ap().rearrange("r (i t) -> r i t", t=TOK)
    out_r = out_d.ap().rearrange("(ncc p) t -> p ncc t", p=P)

    with tile.TileContext(nc) as tc:
        with (
            tc.tile_pool(name="const", bufs=1) as const,
            tc.tile_pool(name="xp", bufs=xp_bufs) as xp,
            tc.tile_pool(name="smp", bufs=2) as smp,
            tc.tile_pool(name="op", bufs=2) as op,
            tc.tile_pool(name="pss", bufs=2, space="PSUM") as pss,
            tc.tile_pool(name="pso", bufs=pso_bufs, space="PSUM") as pso,
        ):
            w1_sb = const.tile([P, KC, D_OUT], FP8)
            w2_sb = const.tile([P, KC, D_OUT], FP8)
            a1_sb = const.tile([P, KC, NR], FP8)
            btr_sb = const.tile([NR // 2, 2, D_OUT], FP8)
            b_sb = const.tile([P, NCC], F32)
            m_sb = const.tile([NR // 2, 2, TOK], BF16)
            wu_sb = const.tile([P, SUP], BF16)

            # Critical-path preloads, ordered so the first supertile's
            # matmuls can start ASAP: a1 + x1[0] feed the s-pass, w1 k-chunks
            # feed the base term-1 matmuls.  Everything later (w2, btr, ...)
            # arrives while term-1 runs.
            if preload_mode == 0:
                nc.sync.dma_start(out=a1_sb[:], in_=a1_r)
            elif preload_mode == 1:
                nc.gpsimd.dma_start(out=a1_sb[:], in_=a1_r)
            else:
                nc.scalar.dma_start(out=a1_sb[:], in_=a1_r)
            kq1 = KC // w1_split
            for k0 in range(0, KC, kq1):
                nc.scalar.dma_start(
                    out=w1_sb[:, k0 : k0 + kq1, :], in_=w1_r[:, k0 : k0 + kq1, :]
                )
            # only k-chunks 0..5 of w2 are consumed (last DR chunk skipped)
            kc2 = KC - 2
            kq2 = kc2 // w2_split
            for k0 in range(0, kc2, kq2):
                nc.scalar.dma_start(
                    out=w2_sb[:, k0 : k0 + kq2, :], in_=w2_r[:, k0 : k0 + kq2, :]
                )
            nc.gpsimd.dma_start(
                out=m_sb[:, :, : TOK // 2], in_=msk_r[:, :, : TOK // 2]
            )
            nc.gpsimd.dma_start(out=btr_sb[:], in_=btr_r)
            nc.gpsimd.dma_start(out=b_sb[:], in_=bias_d.ap())
            nc.gpsimd.dma_start(
                out=m_sb[:, :, TOK // 2 :], in_=msk_r[:, :, TOK // 2 :]
            )

            # PE warmup: ramp the p-state model during the preload DMAs so
            # the first real matmuls run at full clock.  Reads a tile we
            # memset first (never consumed downstream).
            if n_warm:
                nc.vector.memset(wu_sb[:], 0.0)
                wu_ps = pso.tile([P, SUP], F32, tag="ops")
                for i in range(n_warm):
                    nc.tensor.matmul(
                        wu_ps[:],
                        wu_sb[:, 0:P],
                        wu_sb[:],
                        start=(i == 0),
                        stop=(i == n_warm - 1),
                    )

            # First and last supertiles run as 256-token halves: the first
            # matmul only needs a quarter-size x DMA (earlier start), and the
            # final ACT + output DMA covers half a supertile (shorter tail).
            sched = []
            if split_first:
                sched += [(0, SUP // 2), (SUP // 2, SUP // 2)]
            else:
                sched += [(0, SUP)]
            sched += [(s * SUP, SUP) for s in range(1, N_SUP - 1)]
            if split_last:
                sched += [
                    ((N_SUP - 1) * SUP, SUP // 2),
                    ((N_SUP - 1) * SUP + SUP // 2, SUP // 2),
                ]
            else:
                sched += [((N_SUP - 1) * SUP, SUP)]
            for si, (t0, sup) in enumerate(sched):
                last = si == len(sched) - 1
                x1_sb = xp.tile([P, KC, SUP], FP8, tag="x1")
                nc.sync.dma_start(
                    out=x1_sb[:, :, :sup], in_=x1_r[:, :, t0 : t0 + sup]
                )
                x2_sb = xp.tile([P, KC, SUP], FP8, tag="x2")
                nc.sync.dma_start(
                    out=x2_sb[:, :, :sup], in_=x2_r[:, :, t0 : t0 + sup]
                )

                # s.T = fused_A @ x.T for this supertile, in two 32-row
                # halves so sm can be laid out [32, 2, SUP] for a DoubleRow
                # delta matmul (rows r and r+32 pair up as the two k-subtiles)
                s_half = []
                for h in range(2):
                    s_ps = pss.tile([NR // 2, SUP], F32, tag=f"sps{h}")
                    s_half.append(s_ps)
                    for kd in range(KD):
                        nc.tensor.matmul(
                            s_ps[:, :sup],
                            a1_sb[:, 2 * kd : 2 * kd + 2, 32 * h : 32 * h + 32],
                            x1_sb[:, 2 * kd : 2 * kd + 2, :sup],
                            start=(kd == 0),
                            stop=(kd == KD - 1),
                            perf_mode=DR,
                        )
                sm_sb = smp.tile([NR // 2, 2, SUP], FP8, tag="sm")
                for h in range(2):
                    nc.vector.tensor_mul(
                        sm_sb[:, h, :sup],
                        s_half[h][:, :sup],
                        m_sb[:, h, t0 : t0 + sup],
                    )

                o_sb = op.tile([P, NCC, SUP], BF16, tag="o")

                # Software-pipelined chunk groups: open chunk j's psum group
                # with its term-1 (w1*x1) matmuls up to PRE chunks ahead of
                # closing group j-PRE with its term-2/3 + delta matmuls.
                # This gives the PE w1-only work to chew on while w2/x2 (and
                # at startup btr/msk) are still in flight.
                o_pss = {}

                def open_group(j):
                    nsl = slice(j * P, (j + 1) * P)
                    o_ps = pso.tile([P, SUP], F32, tag="ops")
                    o_pss[j] = o_ps
                    for kd in range(KD):
                        nc.tensor.matmul(
                            o_ps[:, :sup],
                            w1_sb[:, 2 * kd : 2 * kd + 2, nsl],
                            x1_sb[:, 2 * kd : 2 * kd + 2, :sup],
                            start=(kd == 0),
                            stop=False,
                            perf_mode=DR,
                        )

                def close_group(j):
                    nsl = slice(j * P, (j + 1) * P)
                    o_ps = o_pss.pop(j)
                    for kd in range(KD):
                        nc.tensor.matmul(
                            o_ps[:, :sup],
                            w1_sb[:, 2 * kd : 2 * kd + 2, nsl],
                            x2_sb[:, 2 * kd : 2 * kd + 2, :sup],
                            start=False,
                            stop=False,
                            perf_mode=DR,
                        )
                    # The last W2 k-chunk is skipped: its contribution is the
                    # W-quantization residual over a quarter of the
                    # contraction (~2e-3 rel err), traded for 8 fewer matmuls
                    # per supertile.  Validated rel err 0.0159 < 2e-2.
                    for kd in range(KD - 1):
                        nc.tensor.matmul(
                            o_ps[:, :sup],
                            w2_sb[:, 2 * kd : 2 * kd + 2, nsl],
                            x1_sb[:, 2 * kd : 2 * kd + 2, :sup],
                            start=False,
                            stop=False,
                            perf_mode=DR,
                        )
                    nc.tensor.matmul(
                        o_ps[:, :sup],
                        btr_sb[:, :, nsl],
                        sm_sb[:, :, :sup],
                        start=False,
                        stop=True,
                        perf_mode=DR,
                    )
                    nc.scalar.activation(
                        o_sb[:, j, :sup],
                        o_ps[:, :sup],
                        mybir.ActivationFunctionType.Identity,
                        bias=b_sb[:, j : j + 1],
                        scale=float(OSCALE),
                    )

                PRE = pre
                for j in range(NCC):
                    open_group(j)
                    if j >= PRE:
                        close_group(j - PRE)
                for j in range(NCC - PRE, NCC):
                    close_group(j)
                if not last:
                    nc.scalar.dma_start(
                        out=out_r[:, :, t0 : t0 + sup], in_=o_sb[:, :, :sup]
                    )
                else:
                    # last supertile: evict per-chunk so the tail is one
                    # chunk long, not the whole supertile
                    for j in range(NCC):
                        nc.sync.dma_start(
                            out=out_r[:, j, t0 : t0 + sup], in_=o_sb[:, j, :sup]
                        )

    nc.compile()
    return nc


_NC_CACHE = None


def _get_nc():
    global _NC_CACHE
    if _NC_CACHE is None:
        _NC_CACHE = build_bass()
    return _NC_CACHE


def _q8(v):
    return np.asarray(v, dtype=E4)


def make_in_maps(x, W, b, lora_A, lora_B, masks):
    x = np.asarray(x, dtype=np.float32)
    W = np.asarray(W, dtype=np.float32)
    b = np.ascontiguousarray(np.asarray(b, dtype=np.float32))
    lora_A = np.asarray(lora_A, dtype=np.float32)
    lora_B = np.asarray(lora_B, dtype=np.float32)
    masks = np.asarray(masks, dtype=np.float32)

    xs = np.ascontiguousarray(x.reshape(B * T, D_IN).T) * np.float32(1.0 / SX)
    x1 = _q8(xs)
    x2 = _q8(xs - x1.astype(np.float32))

    ws = np.ascontiguousarray(W.T) * np.float32(1.0 / SW)  # [D_IN, D_OUT]
    w1 = _q8(ws)
    w2 = _q8(ws - w1.astype(np.float32))

    A_flat = lora_A.reshape(NR, D_IN)
    a1 = _q8(np.ascontiguousarray(A_flat.T) * np.float32(1.0 / SA))

    B_flat = lora_B.transpose(1, 0, 2).reshape(D_OUT, NR)
    btr_q = _q8(B_flat.T * np.float32(1.0 / SB))  # [NR, D_OUT] fp8
    # DoubleRow pairing: rows r and r+32 become the two k-subtiles
    btr = np.ascontiguousarray(
        btr_q.reshape(2, NR // 2, D_OUT).transpose(1, 0, 2).reshape(
            NR // 2, 2 * D_OUT
        )
    )

    bias = np.ascontiguousarray(b.reshape(NCC, P).T)  # [P, NCC]

    m_full = masks[..., 0].reshape(N_ADAPT, B * T) * np.float32(MSCALE)
    m_exp = np.repeat(m_full, R, axis=0).astype(ml_dtypes.bfloat16)  # [NR, BT]

    in_maps = []
    for c in range(N_CORES):
        sl = slice(c * TOK, (c + 1) * TOK)
        in_maps.append(
            {
                "x1": np.ascontiguousarray(x1[:, sl]),
                "x2": np.ascontiguousarray(x2[:, sl]),
                "w1": w1,
                "w2": w2,
                "a1": a1,
                "btr": btr,
                "bias": bias,
                "msk": np.ascontiguousarray(
                    m_exp[:, sl]
                    .reshape(2, NR // 2, TOK)
                    .transpose(1, 0, 2)
                    .reshape(NR // 2, 2 * TOK)
                ),
            }
        )
    return in_maps


def kernel(x, W, b, lora_A, lora_B, masks):
    nc = _get_nc()
    in_maps = make_in_maps(x, W, b, lora_A, lora_B, masks)
    res = run_bass_kernel_spmd(nc, in_maps, core_ids=list(range(N_CORES)))
    out = np.empty((B * T, D_OUT), dtype=np.float32)
    for c, r in enumerate(res.results):
        out[c * TOK : (c + 1) * TOK, :] = r["out"].astype(np.float32).T
    return out.reshape(B, T, D_OUT)


# revision 45
# speedup vs baseline: 1.6726x; 1.0345x over previous
"""Routed-LoRA linear layer (moe_routing) on 8 trn2 NeuronCores.

Math (per token t):
  out[t, :] = W @ x[t] + b + 2.0 * sum_n mask[n, t] * (B_n @ (A_n @ x[t]))

Strategy (v2, split-fp8):
  - Data-parallel over B*T = 65536 tokens: 8192 tokens per core.
  - Output computed TRANSPOSED on-device: out.T [D_OUT, TOK], so the bias is
    per-partition and the Activation engine can do the fused
    `out = psum*scale + bias` eviction (DVE stays nearly idle).
  - Base matmul in split fp8 (e4m3) with DoubleRow perf mode:
      x/sx   = x1 + x2   (x2 quantizes the residual at the same scale)
      W.T/sw = W1 + W2
      x@W.T/(sx*sw) ~= W1.T'x1 + W2.T'x1 + W1.T'x2   (x2@W2 dropped)
    DoubleRow contracts 2 k-subtiles (K=256) per matmul at 0.5 cycles/row.
  - LoRA s-pass s = A@x.T from (A1, x1) fp8 DoubleRow; mask multiply on DVE
    with all scale constants folded into the mask host-side; delta matmul in
    fp32r accumulated into the same PSUM bank as the base terms.
  - Output in bf16 (host casts back to fp32): halves output DMA traffic.
    Measured end-to-end max rel err ~7.8e-3 (budget 2e-2).
"""

import numpy as np
import ml_dtypes

import concourse.bass as bass
from concourse import bacc
import concourse.mybir as mybir
import concourse.tile as tile
from concourse.bass_utils import run_bass_kernel_spmd

N_CORES = 8
B, T = 8, 8192
D_IN = 1024
D_OUT = 1024
N_ADAPT, R = 4, 16
NR = N_ADAPT * R  # 64
SCALING = 32.0 / 16.0

TOK = B * T // N_CORES  # 8192 tokens per core
SUP = 512               # tokens per supertile
N_SUP = TOK // SUP      # 16
P = 128
KC = D_IN // P          # 8 contraction chunks of 128
KD = KC // 2            # 4 DoubleRow chunks of 256
NCC = D_OUT // P        # 8 output-row chunks of 128

SX = 2.0 ** -5          # x scale
SW = 2.0 ** -10         # W scale
SA = 2.0 ** -10         # lora_A scale
SB = 2.0 ** -11         # lora_B scale
OSCALE = SX * SW        # psum -> out scale
MSCALE = SCALING * SA * SB / SW  # folded into the routing mask (2^-10, bf16-exact)

F32 = mybir.dt.float32
F32R = mybir.dt.float32r
BF16 = mybir.dt.bfloat16
FP8 = mybir.dt.float8e4
DR = mybir.MatmulPerfMode.DoubleRow
E4 = ml_dtypes.float8_e4m3


def build_bass(n_warm=4, pre=1, pso_bufs=4, w1_split=4, w2_split=2, split_first=False, split_last=False, preload_mode=0, xp_bufs=3):
    nc = bacc.Bacc(
        "TRN2", target_bir_lowering=False, debug=False, num_devices=N_CORES
    )

    x1_d = nc.dram_tensor("x1", [D_IN, TOK], FP8, kind="ExternalInput")
    x2_d = nc.dram_tensor("x2", [D_IN, TOK], FP8, kind="ExternalInput")
    w1_d = nc.dram_tensor("w1", [D_IN, D_OUT], FP8, kind="ExternalInput")
    w2_d = nc.dram_tensor("w2", [D_IN, D_OUT], FP8, kind="ExternalInput")
    a1_d = nc.dram_tensor("a1", [D_IN, NR], FP8, kind="ExternalInput")
    btr_d = nc.dram_tensor("btr", [NR // 2, 2 * D_OUT], FP8, kind="ExternalInput")
    bias_d = nc.dram_tensor("bias", [P, NCC], F32, kind="ExternalInput")
    msk_d = nc.dram_tensor("msk", [NR, TOK], BF16, kind="ExternalInput")
    out_d = nc.dram_tensor("out", [D_OUT, TOK], BF16, kind="ExternalOutput")

    x1_r = x1_d.ap().rearrange("(kc p) t -> p kc t", p=P)
    x2_r = x2_d.ap().rearrange("(kc p) t -> p kc t", p=P)
    w1_r = w1_d.ap().rearrange("(kc p) n -> p kc n", p=P)
    w2_r = w2_d.ap().rearrange("(kc p) n -> p kc n", p=P)
    a1_r = a1_d.ap().rearrange("(kc p) j -> p kc j", p=P)
    btr_r = btr_d.ap().rearrange("r (i n) -> r i n", n=D_OUT)
    out_r = out_d.ap().rearrange("(ncc p) t -> p ncc t", p=P)

    with tile.TileContext(nc) as tc:
        with (
            tc.tile_pool(name="const", bufs=1) as const,
            tc.tile_pool(name="xp", bufs=xp_bufs) as xp,
            tc.tile_pool(name="smp", bufs=2) as smp,
            tc.tile_pool(name="sm64p", bufs=2) as sm64p,
            tc.tile_pool(name="op", bufs=2) as op,
            tc.tile_pool(name="pss", bufs=2, space="PSUM") as pss,
            tc.tile_pool(name="pso", bufs=pso_bufs, space="PSUM") as pso,
        ):
            w1_sb = const.tile([P, KC, D_OUT], FP8)
            w2_sb = const.tile([P, KC, D_OUT], FP8)
            a1_sb = const.tile([P, KC, NR], FP8)
            btr_sb = const.tile([NR // 2, 2, D_OUT], FP8)
            b_sb = const.tile([P, NCC], F32)
            m_sb = const.tile([NR, TOK], BF16)
            wu_sb = const.tile([P, SUP], BF16)

            # Critical-path preloads, ordered so the first supertile's
            # matmuls can start ASAP: a1 + x1[0] feed the s-pass, w1 k-chunks
            # feed the base term-1 matmuls.  Everything later (w2, btr, ...)
            # arrives while term-1 runs.
            kq1 = KC // w1_split
            for k0 in range(0, KC, kq1):
                nc.sync.dma_start(
                    out=w1_sb[:, k0 : k0 + kq1, :], in_=w1_r[:, k0 : k0 + kq1, :]
                )
            # only k-chunks 0..5 of w2 are consumed (last DR chunk skipped)
            kc2 = KC - 2
            kq2 = kc2 // w2_split
            for k0 in range(0, kc2, kq2):
                nc.sync.dma_start(
                    out=w2_sb[:, k0 : k0 + kq2, :], in_=w2_r[:, k0 : k0 + kq2, :]
                )
            nc.gpsimd.dma_start(
                out=m_sb[:, : TOK // 2], in_=msk_d.ap()[:, : TOK // 2]
            )
            nc.gpsimd.dma_start(out=btr_sb[:], in_=btr_r)
            nc.gpsimd.dma_start(out=b_sb[:], in_=bias_d.ap())
            nc.gpsimd.dma_start(
                out=m_sb[:, TOK // 2 :], in_=msk_d.ap()[:, TOK // 2 :]
            )

            # PE warmup: ramp the p-state model during the preload DMAs so
            # the first real matmuls run at full clock.  Reads a tile we
            # memset first (never consumed downstream).
            if n_warm:
                nc.vector.memset(wu_sb[:], 0.0)
                wu_ps = pso.tile([P, SUP], F32, tag="ops")
                for i in range(n_warm):
                    nc.tensor.matmul(
                        wu_ps[:],
                        wu_sb[:, 0:P],
                        wu_sb[:],
                        start=(i == 0),
                        stop=(i == n_warm - 1),
                    )

            # First and last supertiles run as 256-token halves: the first
            # matmul only needs a quarter-size x DMA (earlier start), and the
            # final ACT + output DMA covers half a supertile (shorter tail).
            sched = []
            if split_first:
                sched += [(0, SUP // 2), (SUP // 2, SUP // 2)]
            else:
                sched += [(0, SUP)]
            sched += [(s * SUP, SUP) for s in range(1, N_SUP - 1)]
            if split_last:
                sched += [
                    ((N_SUP - 1) * SUP, SUP // 2),
                    ((N_SUP - 1) * SUP + SUP // 2, SUP // 2),
                ]
            else:
                sched += [((N_SUP - 1) * SUP, SUP)]
            x1_tiles = {}
            x2_tiles = {}
            sm_tiles = {}

            def emit_xdma(nsi):
                t0n, supn = sched[nsi]
                x1t = xp.tile([P, KC, SUP], FP8, tag="x1")
                nc.scalar.dma_start(
                    out=x1t[:, :, :supn], in_=x1_r[:, :, t0n : t0n + supn]
                )
                x1_tiles[nsi] = x1t
                if nsi == 0:
                    # a1 rides right behind x1[0]: the first PE work
                    # (term-1 opens) only needs x1 + w1, so x1 goes first
                    nc.scalar.dma_start(out=a1_sb[:], in_=a1_r)
                x2t = xp.tile([P, KC, SUP], FP8, tag="x2")
                nc.scalar.dma_start(
                    out=x2t[:, :, :supn], in_=x2_r[:, :, t0n : t0n + supn]
                )
                x2_tiles[nsi] = x2t

            def do_spass(nsi):
                # s.T = fused_A @ x.T for supertile nsi as one [64, sup]
                # pass (4 DoubleRow matmuls); DVE mask-multiplies into fp8
                # and a DVE + GPSIMD copy pair repacks into the [32, 2, sup]
                # DoubleRow pairing (rows r and r+32 are the k-subtiles).
                # Scheduled one supertile ahead so the repack latency stays
                # off the delta matmuls' critical path.
                t0n, supn = sched[nsi]
                x1t = x1_tiles[nsi]
                s_ps = pss.tile([NR, SUP], F32, tag="sps")
                for kd in range(KD):
                    nc.tensor.matmul(
                        s_ps[:, :supn],
                        a1_sb[:, 2 * kd : 2 * kd + 2, :],
                        x1t[:, 2 * kd : 2 * kd + 2, :supn],
                        start=(kd == 0),
                        stop=(kd == KD - 1),
                        perf_mode=DR,
                    )
                sm64 = sm64p.tile([NR, SUP], FP8, tag="sm64")
                nc.vector.tensor_mul(
                    sm64[:, :supn], s_ps[:, :supn], m_sb[:, t0n : t0n + supn]
                )
                smt = smp.tile([NR // 2, 2, SUP], FP8, tag="sm")
                sm_tiles[nsi] = smt
                nc.vector.tensor_copy(smt[:, 0, :supn], sm64[0:32, :supn])
                nc.gpsimd.tensor_copy(smt[:, 1, :supn], sm64[32:64, :supn])

            emit_xdma(0)
            for si, (t0, sup) in enumerate(sched):
                last = si == len(sched) - 1
                if not last:
                    emit_xdma(si + 1)
                x1_sb = x1_tiles[si]
                x2_sb = x2_tiles[si]

                o_sb = op.tile([P, NCC, SUP], BF16, tag="o")

                # Software-pipelined chunk groups: open chunk j's psum group
                # with its term-1 (w1*x1) matmuls up to PRE chunks ahead of
                # closing group j-PRE with its term-2/3 + delta matmuls.
                # This gives the PE w1-only work to chew on while w2/x2 (and
                # at startup btr/msk) are still in flight.
                o_pss = {}

                def open_group(j):
                    nsl = slice(j * P, (j + 1) * P)
                    o_ps = pso.tile([P, SUP], F32, tag="ops")
                    o_pss[j] = o_ps
                    for kd in range(KD):
                        nc.tensor.matmul(
                            o_ps[:, :sup],
                            w1_sb[:, 2 * kd : 2 * kd + 2, nsl],
                            x1_sb[:, 2 * kd : 2 * kd + 2, :sup],
                            start=(kd == 0),
                            stop=False,
                            perf_mode=DR,
                        )

                def close_group(j):
                    nsl = slice(j * P, (j + 1) * P)
                    o_ps = o_pss.pop(j)
                    for kd in range(KD):
                        nc.tensor.matmul(
                            o_ps[:, :sup],
                            w1_sb[:, 2 * kd : 2 * kd + 2, nsl],
                            x2_sb[:, 2 * kd : 2 * kd + 2, :sup],
                            start=False,
                            stop=False,
                            perf_mode=DR,
                        )
                    # The last W2 k-chunk is skipped: its contribution is the
                    # W-quantization residual over a quarter of the
                    # contraction (~2e-3 rel err), traded for 8 fewer matmuls
                    # per supertile.  Validated rel err 0.0159 < 2e-2.
                    for kd in range(KD - 1):
                        nc.tensor.matmul(
                            o_ps[:, :sup],
                            w2_sb[:, 2 * kd : 2 * kd + 2, nsl],
                            x1_sb[:, 2 * kd : 2 * kd + 2, :sup],
                            start=False,
                            stop=False,
                            perf_mode=DR,
                        )
                    nc.tensor.matmul(
                        o_ps[:, :sup],
                        btr_sb[:, :, nsl],
                        sm_tiles[si][:, :, :sup],
                        start=False,
                        stop=True,
                        perf_mode=DR,
                    )
                    nc.scalar.activation(
                        o_sb[:, j, :sup],
                        o_ps[:, :sup],
                        mybir.ActivationFunctionType.Identity,
                        bias=b_sb[:, j : j + 1],
                        scale=float(OSCALE),
                    )

                PRE = pre
                order = []
                for j in range(NCC):
                    order.append(("open", j))
                    if j >= PRE:
                        order.append(("close", j - PRE))
                for j in range(NCC - PRE, NCC):
                    order.append(("close", j))
                # s-pass for the NEXT supertile runs early in this one; on
                # the first supertile its own s-pass runs first (after the
                # initial opens, since it waits on a1 which lands after
                # x1[0]), with the next supertile's a few groups later.
                if not last:
                    order.insert(PRE + 1, ("spass", si + 1))
                if si == 0:
                    order.insert(PRE + 1, ("spass", 0))
                for kind, j in order:
                    if kind == "open":
                        open_group(j)
                    elif kind == "close":
                        close_group(j)
                    else:
                        do_spass(j)
                if not last:
                    nc.scalar.dma_start(
                        out=out_r[:, :, t0 : t0 + sup], in_=o_sb[:, :, :sup]
                    )
                else:
                    # last supertile: evict per-chunk so the tail is one
                    # chunk long, not the whole supertile
                    for j in range(NCC):
                        nc.sync.dma_start(
                            out=out_r[:, j, t0 : t0 + sup], in_=o_sb[:, j, :sup]
                        )

    nc.compile()
    return nc


_NC_CACHE = None


def _get_nc():
    global _NC_CACHE
    if _NC_CACHE is None:
        _NC_CACHE = build_bass()
    return _NC_CACHE


def _q8(v):
    return np.asarray(v, dtype=E4)


def make_in_maps(x, W, b, lora_A, lora_B, masks):
    x = np.asarray(x, dtype=np.float32)
    W = np.asarray(W, dtype=np.float32)
    b = np.ascontiguousarray(np.asarray(b, dtype=np.float32))
    lora_A = np.asarray(lora_A, dtype=np.float32)
    lora_B = np.asarray(lora_B, dtype=np.float32)
    masks = np.asarray(masks, dtype=np.float32)

    xs = np.ascontiguousarray(x.reshape(B * T, D_IN).T) * np.float32(1.0 / SX)
    x1 = _q8(xs)
    x2 = _q8(xs - x1.astype(np.float32))

    ws = np.ascontiguousarray(W.T) * np.float32(1.0 / SW)  # [D_IN, D_OUT]
    w1 = _q8(ws)
    w2 = _q8(ws - w1.astype(np.float32))

    A_flat = lora_A.reshape(NR, D_IN)
    a1 = _q8(np.ascontiguousarray(A_flat.T) * np.float32(1.0 / SA))

    B_flat = lora_B.transpose(1, 0, 2).reshape(D_OUT, NR)
    btr_q = _q8(B_flat.T * np.float32(1.0 / SB))  # [NR, D_OUT] fp8
    # DoubleRow pairing: rows r and r+32 become the two k-subtiles
    btr = np.ascontiguousarray(
        btr_q.reshape(2, NR // 2, D_OUT).transpose(1, 0, 2).reshape(
            NR // 2, 2 * D_OUT
        )
    )

    bias = np.ascontiguousarray(b.reshape(NCC, P).T)  # [P, NCC]

    m_full = masks[..., 0].reshape(N_ADAPT, B * T) * np.float32(MSCALE)
    m_exp = np.repeat(m_full, R, axis=0).astype(ml_dtypes.bfloat16)  # [NR, BT]

    in_maps = []
    for c in range(N_CORES):
        sl = slice(c * TOK, (c + 1) * TOK)
        in_maps.append(
            {
                "x1": np.ascontiguousarray(x1[:, sl]),
                "x2": np.ascontiguousarray(x2[:, sl]),
                "w1": w1,
                "w2": w2,
                "a1": a1,
                "btr": btr,
                "bias": bias,
                "msk": np.ascontiguousarray(m_exp[:, sl]),
            }
        )
    return in_maps


def kernel(x, W, b, lora_A, lora_B, masks):
    nc = _get_nc()
    in_maps = make_in_maps(x, W, b, lora_A, lora_B, masks)
    res = run_bass_kernel_spmd(nc, in_maps, core_ids=list(range(N_CORES)))
    out = np.empty((B * T, D_OUT), dtype=np.float32)
    for c, r in enumerate(res.results):
        out[c * TOK : (c + 1) * TOK, :] = r["out"].astype(np.float32).T
    return out.reshape(B, T, D_OUT)
